# revision 23
# baseline (speedup 1.0000x reference)
"""Trainium2 Bass kernel for single-head causal attention.

Problem: B=8, T=2048, C=1024, HS=64
  q = x_q @ Wq; k = x_kv @ Wk; v = x_kv @ Wv        (all [B,T,HS])
  wei = softmax(mask(q @ k.T * C**-0.5))            ([B,T,T], causal)
  out = wei @ v                                      ([B,T,HS])

Sharding: data-parallel over batch B across 8 cores (1 batch element/core).

Per-core design (PE-dense schedule):
  - transposed layout: host pre-transposes x; contraction dim C on SBUF
    partitions; host layout is partition-major so DMA lines are contiguous.
  - HAM warm-up burst (no DMA dependency) opens the PE clock-gate early.
  - projections: qT (duplicated to both partition halves via a [Wq|Wq]
    stationary), kvT = [Wk|Wv].T @ x_kv; kT copied to partitions 64:128
    (k2) by an SBUF-SBUF DMA.
  - scores ROW-PACKED: two K=64 matmuls run concurrently in the two
    64-row groups of the PE array; pairs land in one [128,2,512] PSUM
    tile; one paired Exp ACTIVATE per two blocks; diagonal masks on DVE.
    The (m2,m3) diagonal pair is restricted to the causal trapezoid.
  - all scores+projections are emitted before/with the PV phase so the
    scalar-engine Exp hides behind PE matmul work; P for all 40 blocks
    is kept in SBUF.
  - PV: out'[65,tq] += v'[tk,65].T @ P[tk,tq]; row 64 = softmax
    denominator via ones-column in v'; diagonal blocks N-restricted.
  - output: raw [65, T] fp32 DMA'd out; normalization + transpose on host.
"""

import sys

sys.path.insert(0, "/opt/trn_rl_repo")

import numpy as np
import ml_dtypes

import concourse.bass as bass
from concourse import bacc
import concourse.mybir as mybir
import concourse.tile as tile
from concourse.bass_utils import run_bass_kernel_spmd
from concourse.masks import make_identity

FP32 = mybir.dt.float32
T, C, HS = 2048, 1024, 64
NSLICE = 4          # tq slices of 512
TS = T // NSLICE    # 512
CK = C // 128       # 8 c-chunks
NJ = T // 128       # 16 tk tiles of 128
SCALE = float(C) ** -0.5
DT = mybir.dt.bfloat16
WARM_MMS = 16       # dummy matmuls at t=0 to flip the HAM clock gate
# p-block storage base index per slice (slice i has 4i+4 blocks)
PBASE = [0, 4, 12, 24]
NP_TOT = 40


def build_bass():
    nc = bacc.Bacc(None, target_bir_lowering=False)
    xq = nc.dram_tensor("xq", [NSLICE, 128, CK, TS], DT, kind="ExternalInput").ap()
    xk = nc.dram_tensor("xk", [NSLICE, 128, CK, TS], DT, kind="ExternalInput").ap()
    # wq holds [Wq | Wq] per chunk (M=128): the q matmul then emits qT
    # duplicated into both partition halves at no extra cost.
    wq = nc.dram_tensor("wq", [128, CK * 128], DT, kind="ExternalInput").ap()
    wkv = nc.dram_tensor("wkv", [128, CK * 2 * HS], DT, kind="ExternalInput").ap()
    oT = nc.dram_tensor("oT", [HS + 1, T], FP32, kind="ExternalOutput").ap()

    with tile.TileContext(nc) as tc:
        with (
            tc.tile_pool(name="singles", bufs=1) as singles,
            tc.tile_pool(name="xpool", bufs=2) as xpool,
            tc.tile_pool(name="ob", bufs=2) as ob,
            tc.tile_pool(name="pp_q", bufs=1, space="PSUM") as pp_q,
            tc.tile_pool(name="pp_kv", bufs=1, space="PSUM") as pp_kv,
            tc.tile_pool(name="pp_st", bufs=2, space="PSUM") as pp_st,
            tc.tile_pool(name="pp_o", bufs=1, space="PSUM") as pp_o,
            tc.tile_pool(name="pp_tr", bufs=1, space="PSUM") as pp_tr,
        ):
            # ---- HAM warm-up (no DMA dependency: feeds off a DVE memset) --
            warm_sb = singles.tile([128, TS], DT)
            nc.vector.memset(warm_sb, 1.0)
            warm_ps = pp_st.tile([128, 2, TS], FP32, tag="st")
            for w in range(WARM_MMS):
                nc.tensor.matmul(
                    warm_ps[:, 0, :],
                    warm_sb[:, 0:128],
                    warm_sb,
                    start=(w == 0),
                    stop=(w == WARM_MMS - 1),
                )

            # ---- weights (first DMAs on the sync queue) ----
            wq_sb = singles.tile([128, CK, 128], DT)
            nc.sync.dma_start(out=wq_sb, in_=wq.rearrange("p (c h) -> p c h", c=CK))
            wkv_sb = singles.tile([128, CK, 2 * HS], DT)
            nc.sync.dma_start(
                out=wkv_sb, in_=wkv.rearrange("p (c h) -> p c h", c=CK)
            )

            ident_dt = singles.tile([128, 128], DT)
            make_identity(nc, ident_dt)

            # diag masks: mask[m][x, y] = 1.0 if y - x >= 128*m else 0.0
            masks = singles.tile([128, 4, TS], DT)
            nc.gpsimd.memset(masks, 1.0)
            for m in range(4):
                nc.gpsimd.affine_select(
                    out=masks[:, m, :],
                    in_=masks[:, m, :],
                    compare_op=mybir.AluOpType.is_ge,
                    fill=0.0,
                    base=-128 * m,
                    pattern=[[1, TS]],
                    channel_multiplier=-1,
                )

            # persistent activations
            qq_sb = singles.tile([128, T], DT)   # qT duplicated in both halves
            kv_sb = singles.tile([128, T], DT)   # rows 0:64 kT, rows 64:128 vT
            k2_sb = singles.tile([128, T], DT)   # rows 64:128 kT (copy)
            p_sb = singles.tile([128, NP_TOT, TS], DT)  # exp(S), all slices
            v_sb = singles.tile([128, NJ, HS + 1], DT)  # v natural + ones col
            ones_f32 = singles.tile([128, 1], FP32)
            nc.vector.memset(ones_f32, 1.0)
            nc.vector.tensor_copy(
                v_sb[:, :, HS : HS + 1], ones_f32.broadcast_to((128, NJ, 1))
            )

            xtiles = {}

            def emit_dma(s):
                h = CK // 2
                tl = {}
                tl["qlo"] = xpool.tile([128, h, TS], DT, tag="xqlo", name="xqlo")
                tl["qhi"] = xpool.tile([128, h, TS], DT, tag="xqhi", name="xqhi")
                tl["klo"] = xpool.tile([128, h, TS], DT, tag="xklo", name="xklo")
                tl["khi"] = xpool.tile([128, h, TS], DT, tag="xkhi", name="xkhi")
                nc.sync.dma_start(out=tl["qlo"], in_=xq[s][:, 0:h])
                nc.sync.dma_start(out=tl["qhi"], in_=xq[s][:, h:CK])
                nc.sync.dma_start(out=tl["klo"], in_=xk[s][:, 0:h])
                nc.sync.dma_start(out=tl["khi"], in_=xk[s][:, h:CK])
                xtiles[s] = tl

            def emit_proj_q(s, c0, c1):
                """q-projection matmuls for chunks [c0, c1) of slice s."""
                h = CK // 2
                tl = xtiles[s]
                for ci in range(c0, c1):
                    xq_t = tl["qlo"] if ci < h else tl["qhi"]
                    nc.tensor.matmul(
                        proj_ps[s][0],
                        wq_sb[:, ci, :],
                        xq_t[:, ci % h, :],
                        start=(ci == 0),
                        stop=(ci == CK - 1),
                    )

            def emit_qq_copy(s):
                t0 = s * TS
                nc.vector.tensor_copy(qq_sb[:, t0 : t0 + TS], proj_ps[s][0])

            def emit_proj_kv(s, c0, c1):
                """kv-projection matmuls for chunks [c0, c1) of slice s."""
                h = CK // 2
                tl = xtiles[s]
                for ci in range(c0, c1):
                    xk_t = tl["klo"] if ci < h else tl["khi"]
                    nc.tensor.matmul(
                        proj_ps[s][1],
                        wkv_sb[:, ci, :],
                        xk_t[:, ci % h, :],
                        start=(ci == 0),
                        stop=(ci == CK - 1),
                    )

            def emit_proj_fin(s):
                """kv copies + k2 shift + v transposes for slice s."""
                t0 = s * TS
                kv_ps = proj_ps[s][1]
                nc.vector.tensor_copy(kv_sb[0:64, t0 : t0 + TS], kv_ps[0:64, :])
                nc.vector.tensor_copy(kv_sb[64:128, t0 : t0 + TS], kv_ps[64:128, :])
                nc.scalar.dma_start(
                    out=k2_sb[64:128, t0 : t0 + TS],
                    in_=kv_sb[0:64, t0 : t0 + TS],
                )
                tr = pp_tr.tile([128, 4, HS], DT, tag="tr", name="tr")
                for jj in range(4):
                    j = 4 * s + jj
                    nc.tensor.transpose(
                        tr[:, jj, :],
                        kv_sb[64:128, j * 128 : (j + 1) * 128],
                        ident_dt[64:128, 64:128],
                    )
                nc.vector.tensor_copy(v_sb[:, 4 * s : 4 * s + 4, 0:HS], tr)

            def emit_score_pair(i, a):
                """Row-packed score pair (j0=2a, j1=2a+1) for tq-slice i.

                The last pair of each slice (m=2,3 diagonal blocks) is
                restricted to the causal trapezoid.
                """
                t0 = i * TS
                j0, j1 = 2 * a, 2 * a + 1
                pb = PBASE[i]
                st = pp_st.tile([128, 2, TS], FP32, tag="st", name="st")
                restricted = j0 == 4 * i + 2  # (m2, m3) pair
                lo0 = 256 if restricted else 0
                lo1 = 384 if restricted else 0
                nc.tensor.matmul(
                    st[:, 0, lo0:TS],
                    kv_sb[0:64, j0 * 128 : (j0 + 1) * 128],
                    qq_sb[0:64, t0 + lo0 : t0 + TS],
                    start=True,
                    stop=True,
                )
                nc.tensor.matmul(
                    st[:, 1, lo1:TS],
                    k2_sb[64:128, j1 * 128 : (j1 + 1) * 128],
                    qq_sb[64:128, t0 + lo1 : t0 + TS],
                    start=True,
                    stop=True,
                )
                if restricted:
                    nc.scalar.activation(
                        out=p_sb[:, pb + j0, lo0:TS],
                        in_=st[:, 0, lo0:TS],
                        func=mybir.ActivationFunctionType.Exp,
                        scale=SCALE,
                    )
                    nc.scalar.activation(
                        out=p_sb[:, pb + j1, lo1:TS],
                        in_=st[:, 1, lo1:TS],
                        func=mybir.ActivationFunctionType.Exp,
                        scale=SCALE,
                    )
                else:
                    nc.scalar.activation(
                        out=p_sb[:, pb + j0 : pb + j0 + 2, :],
                        in_=st,
                        func=mybir.ActivationFunctionType.Exp,
                        scale=SCALE,
                    )
                for j, lo in ((j0, lo0), (j1, lo1)):
                    if j >= 4 * i:
                        m = j - 4 * i
                        nc.vector.tensor_mul(
                            p_sb[:, pb + j, lo:TS], p_sb[:, pb + j, lo:TS],
                            masks[:, m, lo:TS],
                        )

            pv_state = {}

            def emit_pv(i, j0, j1):
                """PV matmuls j in [j0, j1) for tq-slice i."""
                nj = 4 * i + 4
                if i not in pv_state:
                    pv_state[i] = pp_o.tile([HS + 1, TS], FP32, tag="o", name="o")
                o_ps = pv_state[i]
                pb = PBASE[i]
                for j in range(j0, j1):
                    m = j - 4 * i
                    lo = 128 * m if m > 0 else 0
                    nc.tensor.matmul(
                        o_ps[:, lo:TS],
                        v_sb[:, j, :],
                        p_sb[:, pb + j, lo:TS],
                        start=(j == 0),
                        stop=(j == nj - 1),
                    )
                if j1 == nj:
                    osb = ob.tile([HS + 1, TS], FP32, tag="ot", name="osb")
                    nc.vector.tensor_copy(osb, o_ps)
                    nc.scalar.dma_start(
                        out=oT[:, i * TS : (i + 1) * TS], in_=osb
                    )

            # ---------------- emission schedule ----------------
            proj_ps = {}
            for s in range(NSLICE):
                proj_ps[s] = (
                    pp_q.tile([128, TS], FP32, tag="q", name=f"q_ps{s}"),
                    pp_kv.tile([128, TS], FP32, tag="kv", name=f"kv_ps{s}"),
                )

            emit_dma(0)
            emit_dma(1)
            emit_proj_q(0, 0, CK)
            emit_qq_copy(0)
            emit_proj_kv(0, 0, CK)
            emit_proj_fin(0)
            # slice 0 scores (2 pairs; both tiles free -> no ACT backlog)
            emit_score_pair(0, 0)
            emit_score_pair(0, 1)
            emit_dma(2)
            emit_proj_q(1, 0, CK)
            emit_qq_copy(1)
            emit_proj_kv(1, 0, CK)
            emit_proj_fin(1)
            emit_dma(3)
            # steady state: >= ~850ns of other PE work woven between score
            # pairs so the paired-Exp ACT cadence (~1.15us) never stalls PE
            emit_score_pair(1, 0)
            emit_proj_q(2, 0, 4)
            emit_score_pair(1, 1)
            emit_proj_q(2, 4, CK)
            emit_qq_copy(2)
            emit_score_pair(1, 2)
            emit_proj_kv(2, 0, 4)
            emit_score_pair(1, 3)
            emit_proj_kv(2, 4, CK)
            emit_proj_fin(2)
            emit_score_pair(2, 0)
            emit_proj_q(3, 0, 4)
            emit_score_pair(2, 1)
            emit_proj_q(3, 4, CK)
            emit_qq_copy(3)
            emit_score_pair(2, 2)
            emit_proj_kv(3, 0, 4)
            emit_score_pair(2, 3)
            emit_proj_kv(3, 4, CK)
            emit_proj_fin(3)
            emit_score_pair(2, 4)
            emit_pv(0, 0, 4)
            emit_score_pair(2, 5)
            emit_pv(1, 0, 4)
            emit_score_pair(3, 0)
            emit_pv(1, 4, 8)
            emit_score_pair(3, 1)
            emit_pv(2, 0, 4)
            emit_score_pair(3, 2)
            emit_pv(2, 4, 8)
            emit_score_pair(3, 3)
            emit_pv(2, 8, 12)
            emit_score_pair(3, 4)
            emit_pv(3, 0, 4)
            emit_score_pair(3, 5)
            emit_pv(3, 4, 8)
            emit_score_pair(3, 6)
            emit_pv(3, 8, 12)
            emit_score_pair(3, 7)
            emit_pv(3, 12, 16)
    nc.compile()
    return nc


_NC_CACHE = {}


def _get_nc():
    key = "v10"
    if key not in _NC_CACHE:
        _NC_CACHE[key] = build_bass()
    return _NC_CACHE[key]


def kernel(x_q, x_kv, Wq, Wk, Wv, _trace=False):
    B = x_q.shape[0]
    assert B == 8 and x_q.shape == (8, T, C)
    hdt = ml_dtypes.bfloat16

    def _swz(w):
        h = w.shape[1]
        return np.ascontiguousarray(
            w.reshape(CK, 128, h).transpose(1, 0, 2).reshape(128, CK * h)
        ).astype(hdt)

    wkv = _swz(np.concatenate([Wk, Wv], axis=1))
    wq = _swz(np.concatenate([Wq, Wq], axis=1))

    # [B, T, C] -> partition-major [B, NSLICE, 128, CK, TS] so the device
    # DMA reads are fully contiguous per partition:
    #   dev[s, p, c, t] == x.T[c*128 + p, s*TS + t]
    def _xdev(x):
        xt = x.transpose(0, 2, 1).reshape(B, CK, 128, NSLICE, TS)
        return np.ascontiguousarray(xt.transpose(0, 3, 2, 1, 4)).astype(hdt)

    xqT = _xdev(x_q)
    xkT = _xdev(x_kv)

    in_maps = [
        {"xq": xqT[b], "xk": xkT[b], "wq": wq, "wkv": wkv} for b in range(B)
    ]
    nc = _get_nc()
    res = run_bass_kernel_spmd(nc, in_maps, core_ids=list(range(B)), trace=_trace)
    # oT: [65, T]; rows 0:64 unnormalized out^T, row 64 softmax denominator
    outs = []
    for r in res.results:
        o = r["oT"]
        outs.append(np.ascontiguousarray((o[0:HS] / o[HS : HS + 1]).T))
    out = np.stack(outs).astype(np.float32)
    if _trace:
        kernel.last_result = res
    return out


# revision 24
# speedup vs baseline: 1.0002x; 1.0002x over previous
"""Trainium2 Bass kernel for single-head causal attention.

Problem: B=8, T=2048, C=1024, HS=64
  q = x_q @ Wq; k = x_kv @ Wk; v = x_kv @ Wv        (all [B,T,HS])
  wei = softmax(mask(q @ k.T * C**-0.5))            ([B,T,T], causal)
  out = wei @ v                                      ([B,T,HS])

Sharding: data-parallel over batch B across 8 cores (1 batch element/core).

Per-core design (PE-dense schedule):
  - transposed layout: host pre-transposes x; contraction dim C on SBUF
    partitions; host layout is partition-major so DMA lines are contiguous.
  - HAM warm-up burst (no DMA dependency) opens the PE clock-gate early.
  - projections: qT (duplicated to both partition halves via a [Wq|Wq]
    stationary), kvT = [Wk|Wv].T @ x_kv; kT copied to partitions 64:128
    (k2) by an SBUF-SBUF DMA.
  - scores ROW-PACKED: two K=64 matmuls run concurrently in the two
    64-row groups of the PE array; pairs land in one [128,2,512] PSUM
    tile; one paired Exp ACTIVATE per two blocks; diagonal masks on DVE.
    The (m2,m3) diagonal pair is restricted to the causal trapezoid.
  - all scores+projections are emitted before/with the PV phase so the
    scalar-engine Exp hides behind PE matmul work; P for all 40 blocks
    is kept in SBUF.
  - PV: out'[65,tq] += v'[tk,65].T @ P[tk,tq]; row 64 = softmax
    denominator via ones-column in v'; diagonal blocks N-restricted.
  - output: raw [65, T] fp32 DMA'd out; normalization + transpose on host.
"""

import sys

sys.path.insert(0, "/opt/trn_rl_repo")

import numpy as np
import ml_dtypes

import concourse.bass as bass
from concourse import bacc
import concourse.mybir as mybir
import concourse.tile as tile
from concourse.bass_utils import run_bass_kernel_spmd
from concourse.masks import make_identity

FP32 = mybir.dt.float32
T, C, HS = 2048, 1024, 64
NSLICE = 4          # tq slices of 512
TS = T // NSLICE    # 512
CK = C // 128       # 8 c-chunks
NJ = T // 128       # 16 tk tiles of 128
SCALE = float(C) ** -0.5
DT = mybir.dt.bfloat16
WARM_MMS = 16       # dummy matmuls at t=0 to flip the HAM clock gate
# p-block storage base index per slice (slice i has 4i+4 blocks)
PBASE = [0, 4, 12, 24]
NP_TOT = 40


def build_bass():
    nc = bacc.Bacc(None, target_bir_lowering=False)
    xq = nc.dram_tensor("xq", [NSLICE, 128, CK, TS], DT, kind="ExternalInput").ap()
    xk = nc.dram_tensor("xk", [NSLICE, 128, CK, TS], DT, kind="ExternalInput").ap()
    # wq holds [Wq | Wq] per chunk (M=128): the q matmul then emits qT
    # duplicated into both partition halves at no extra cost.
    wq = nc.dram_tensor("wq", [128, CK * 128], DT, kind="ExternalInput").ap()
    wkv = nc.dram_tensor("wkv", [128, CK * 2 * HS], DT, kind="ExternalInput").ap()
    oT = nc.dram_tensor("oT", [HS + 1, T], FP32, kind="ExternalOutput").ap()

    with tile.TileContext(nc) as tc:
        with (
            tc.tile_pool(name="singles", bufs=1) as singles,
            tc.tile_pool(name="xpool", bufs=2) as xpool,
            tc.tile_pool(name="ob", bufs=2) as ob,
            tc.tile_pool(name="pp_q", bufs=1, space="PSUM") as pp_q,
            tc.tile_pool(name="pp_kv", bufs=1, space="PSUM") as pp_kv,
            tc.tile_pool(name="pp_st", bufs=2, space="PSUM") as pp_st,
            tc.tile_pool(name="pp_o", bufs=1, space="PSUM") as pp_o,
            tc.tile_pool(name="pp_tr", bufs=1, space="PSUM") as pp_tr,
        ):
            # ---- HAM warm-up (no DMA dependency: feeds off a DVE memset) --
            warm_sb = singles.tile([128, TS], DT)
            nc.vector.memset(warm_sb, 1.0)
            warm_ps = pp_st.tile([128, 2, TS], FP32, tag="st")
            for w in range(WARM_MMS):
                nc.tensor.matmul(
                    warm_ps[:, 0, :],
                    warm_sb[:, 0:128],
                    warm_sb,
                    start=(w == 0),
                    stop=(w == WARM_MMS - 1),
                )

            # ---- weights (first DMAs on the sync queue) ----
            wq_sb = singles.tile([128, CK, 128], DT)
            nc.sync.dma_start(out=wq_sb, in_=wq.rearrange("p (c h) -> p c h", c=CK))
            wkv_sb = singles.tile([128, CK, 2 * HS], DT)
            nc.sync.dma_start(
                out=wkv_sb, in_=wkv.rearrange("p (c h) -> p c h", c=CK)
            )

            ident_dt = singles.tile([128, 128], DT)
            make_identity(nc, ident_dt)

            # diag masks: mask[m][x, y] = 1.0 if y - x >= 128*m else 0.0
            masks = singles.tile([128, 4, TS], DT)
            nc.gpsimd.memset(masks, 1.0)
            for m in range(4):
                nc.gpsimd.affine_select(
                    out=masks[:, m, :],
                    in_=masks[:, m, :],
                    compare_op=mybir.AluOpType.is_ge,
                    fill=0.0,
                    base=-128 * m,
                    pattern=[[1, TS]],
                    channel_multiplier=-1,
                )

            # persistent activations
            qq_sb = singles.tile([128, T], DT)   # qT duplicated in both halves
            kv_sb = singles.tile([128, T], DT)   # rows 0:64 kT, rows 64:128 vT
            k2_sb = singles.tile([128, T], DT)   # rows 64:128 kT (copy)
            p_sb = singles.tile([128, NP_TOT, TS], DT)  # exp(S), all slices
            v_sb = singles.tile([128, NJ, HS + 1], DT)  # v natural + ones col
            ones_f32 = singles.tile([128, 1], FP32)
            nc.vector.memset(ones_f32, 1.0)
            nc.vector.tensor_copy(
                v_sb[:, :, HS : HS + 1], ones_f32.broadcast_to((128, NJ, 1))
            )

            xtiles = {}

            def emit_dma(s):
                h = CK // 2
                tl = {}
                tl["qlo"] = xpool.tile([128, h, TS], DT, tag="xqlo", name="xqlo")
                tl["qhi"] = xpool.tile([128, h, TS], DT, tag="xqhi", name="xqhi")
                tl["klo"] = xpool.tile([128, h, TS], DT, tag="xklo", name="xklo")
                tl["khi"] = xpool.tile([128, h, TS], DT, tag="xkhi", name="xkhi")
                # 256KB chunks: the sync engine's ~600ns issue rate keeps
                # at most ~2 transfers pending, so data arrives in
                # consumption order instead of round-robin across transfers
                for c2 in range(0, h, 2):
                    nc.sync.dma_start(
                        out=tl["qlo"][:, c2 : c2 + 2], in_=xq[s][:, c2 : c2 + 2]
                    )
                for c2 in range(0, h, 2):
                    nc.sync.dma_start(
                        out=tl["qhi"][:, c2 : c2 + 2],
                        in_=xq[s][:, h + c2 : h + c2 + 2],
                    )
                for c2 in range(0, h, 2):
                    nc.sync.dma_start(
                        out=tl["klo"][:, c2 : c2 + 2], in_=xk[s][:, c2 : c2 + 2]
                    )
                for c2 in range(0, h, 2):
                    nc.sync.dma_start(
                        out=tl["khi"][:, c2 : c2 + 2],
                        in_=xk[s][:, h + c2 : h + c2 + 2],
                    )
                xtiles[s] = tl

            def emit_proj_q(s, c0, c1):
                """q-projection matmuls for chunks [c0, c1) of slice s."""
                h = CK // 2
                tl = xtiles[s]
                for ci in range(c0, c1):
                    xq_t = tl["qlo"] if ci < h else tl["qhi"]
                    nc.tensor.matmul(
                        proj_ps[s][0],
                        wq_sb[:, ci, :],
                        xq_t[:, ci % h, :],
                        start=(ci == 0),
                        stop=(ci == CK - 1),
                    )

            def emit_qq_copy(s):
                t0 = s * TS
                nc.vector.tensor_copy(qq_sb[:, t0 : t0 + TS], proj_ps[s][0])

            def emit_proj_kv(s, c0, c1):
                """kv-projection matmuls for chunks [c0, c1) of slice s."""
                h = CK // 2
                tl = xtiles[s]
                for ci in range(c0, c1):
                    xk_t = tl["klo"] if ci < h else tl["khi"]
                    nc.tensor.matmul(
                        proj_ps[s][1],
                        wkv_sb[:, ci, :],
                        xk_t[:, ci % h, :],
                        start=(ci == 0),
                        stop=(ci == CK - 1),
                    )

            def emit_proj_fin(s):
                """kv copies + k2 shift + v transposes for slice s."""
                t0 = s * TS
                kv_ps = proj_ps[s][1]
                nc.vector.tensor_copy(kv_sb[0:64, t0 : t0 + TS], kv_ps[0:64, :])
                nc.vector.tensor_copy(kv_sb[64:128, t0 : t0 + TS], kv_ps[64:128, :])
                nc.scalar.dma_start(
                    out=k2_sb[64:128, t0 : t0 + TS],
                    in_=kv_sb[0:64, t0 : t0 + TS],
                )
                tr = pp_tr.tile([128, 4, HS], DT, tag="tr", name="tr")
                for jj in range(4):
                    j = 4 * s + jj
                    nc.tensor.transpose(
                        tr[:, jj, :],
                        kv_sb[64:128, j * 128 : (j + 1) * 128],
                        ident_dt[64:128, 64:128],
                    )
                nc.vector.tensor_copy(v_sb[:, 4 * s : 4 * s + 4, 0:HS], tr)

            def emit_score_pair(i, a):
                """Row-packed score pair (j0=2a, j1=2a+1) for tq-slice i.

                The last pair of each slice (m=2,3 diagonal blocks) is
                restricted to the causal trapezoid.
                """
                t0 = i * TS
                j0, j1 = 2 * a, 2 * a + 1
                pb = PBASE[i]
                st = pp_st.tile([128, 2, TS], FP32, tag="st", name="st")
                restricted = j0 == 4 * i + 2  # (m2, m3) pair
                lo0 = 256 if restricted else 0
                lo1 = 384 if restricted else 0
                nc.tensor.matmul(
                    st[:, 0, lo0:TS],
                    kv_sb[0:64, j0 * 128 : (j0 + 1) * 128],
                    qq_sb[0:64, t0 + lo0 : t0 + TS],
                    start=True,
                    stop=True,
                )
                nc.tensor.matmul(
                    st[:, 1, lo1:TS],
                    k2_sb[64:128, j1 * 128 : (j1 + 1) * 128],
                    qq_sb[64:128, t0 + lo1 : t0 + TS],
                    start=True,
                    stop=True,
                )
                if restricted:
                    nc.scalar.activation(
                        out=p_sb[:, pb + j0, lo0:TS],
                        in_=st[:, 0, lo0:TS],
                        func=mybir.ActivationFunctionType.Exp,
                        scale=SCALE,
                    )
                    nc.scalar.activation(
                        out=p_sb[:, pb + j1, lo1:TS],
                        in_=st[:, 1, lo1:TS],
                        func=mybir.ActivationFunctionType.Exp,
                        scale=SCALE,
                    )
                else:
                    nc.scalar.activation(
                        out=p_sb[:, pb + j0 : pb + j0 + 2, :],
                        in_=st,
                        func=mybir.ActivationFunctionType.Exp,
                        scale=SCALE,
                    )
                for j, lo in ((j0, lo0), (j1, lo1)):
                    if j >= 4 * i:
                        m = j - 4 * i
                        nc.vector.tensor_mul(
                            p_sb[:, pb + j, lo:TS], p_sb[:, pb + j, lo:TS],
                            masks[:, m, lo:TS],
                        )

            pv_state = {}

            def emit_pv(i, j0, j1):
                """PV matmuls j in [j0, j1) for tq-slice i."""
                nj = 4 * i + 4
                if i not in pv_state:
                    pv_state[i] = pp_o.tile([HS + 1, TS], FP32, tag="o", name="o")
                o_ps = pv_state[i]
                pb = PBASE[i]
                for j in range(j0, j1):
                    m = j - 4 * i
                    lo = 128 * m if m > 0 else 0
                    nc.tensor.matmul(
                        o_ps[:, lo:TS],
                        v_sb[:, j, :],
                        p_sb[:, pb + j, lo:TS],
                        start=(j == 0),
                        stop=(j == nj - 1),
                    )
                if j1 == nj:
                    osb = ob.tile([HS + 1, TS], FP32, tag="ot", name="osb")
                    nc.vector.tensor_copy(osb, o_ps)
                    nc.sync.dma_start(
                        out=oT[:, i * TS : (i + 1) * TS], in_=osb
                    )

            # ---------------- emission schedule ----------------
            proj_ps = {}
            for s in range(NSLICE):
                proj_ps[s] = (
                    pp_q.tile([128, TS], FP32, tag="q", name=f"q_ps{s}"),
                    pp_kv.tile([128, TS], FP32, tag="kv", name=f"kv_ps{s}"),
                )

            emit_dma(0)
            emit_dma(1)
            emit_proj_q(0, 0, CK)
            emit_qq_copy(0)
            emit_proj_kv(0, 0, CK)
            emit_proj_fin(0)
            # slice 0 scores (2 pairs; both tiles free -> no ACT backlog)
            emit_score_pair(0, 0)
            emit_score_pair(0, 1)
            emit_dma(2)
            emit_proj_q(1, 0, CK)
            emit_qq_copy(1)
            emit_proj_kv(1, 0, CK)
            emit_proj_fin(1)
            emit_dma(3)
            # steady state: >= ~850ns of other PE work woven between score
            # pairs so the paired-Exp ACT cadence (~1.15us) never stalls PE
            emit_score_pair(1, 0)
            emit_proj_q(2, 0, 4)
            emit_score_pair(1, 1)
            emit_proj_q(2, 4, CK)
            emit_qq_copy(2)
            emit_score_pair(1, 2)
            emit_proj_kv(2, 0, 4)
            emit_score_pair(1, 3)
            emit_proj_kv(2, 4, CK)
            emit_proj_fin(2)
            emit_score_pair(2, 0)
            emit_proj_q(3, 0, 4)
            emit_score_pair(2, 1)
            emit_proj_q(3, 4, CK)
            emit_qq_copy(3)
            emit_score_pair(2, 2)
            emit_proj_kv(3, 0, 4)
            emit_score_pair(2, 3)
            emit_proj_kv(3, 4, CK)
            emit_proj_fin(3)
            emit_score_pair(2, 4)
            emit_pv(0, 0, 4)
            emit_score_pair(2, 5)
            emit_pv(1, 0, 4)
            emit_score_pair(3, 0)
            emit_pv(1, 4, 8)
            emit_score_pair(3, 1)
            emit_pv(2, 0, 4)
            emit_score_pair(3, 2)
            emit_pv(2, 4, 8)
            emit_score_pair(3, 3)
            emit_pv(2, 8, 12)
            emit_score_pair(3, 4)
            emit_pv(3, 0, 4)
            emit_score_pair(3, 5)
            emit_pv(3, 4, 8)
            emit_score_pair(3, 6)
            emit_pv(3, 8, 12)
            emit_score_pair(3, 7)
            emit_pv(3, 12, 16)
    nc.compile()
    return nc


_NC_CACHE = {}


def _get_nc():
    key = "v11"
    if key not in _NC_CACHE:
        _NC_CACHE[key] = build_bass()
    return _NC_CACHE[key]


def kernel(x_q, x_kv, Wq, Wk, Wv, _trace=False):
    B = x_q.shape[0]
    assert B == 8 and x_q.shape == (8, T, C)
    hdt = ml_dtypes.bfloat16

    def _swz(w):
        h = w.shape[1]
        return np.ascontiguousarray(
            w.reshape(CK, 128, h).transpose(1, 0, 2).reshape(128, CK * h)
        ).astype(hdt)

    wkv = _swz(np.concatenate([Wk, Wv], axis=1))
    wq = _swz(np.concatenate([Wq, Wq], axis=1))

    # [B, T, C] -> partition-major [B, NSLICE, 128, CK, TS] so the device
    # DMA reads are fully contiguous per partition:
    #   dev[s, p, c, t] == x.T[c*128 + p, s*TS + t]
    def _xdev(x):
        xt = x.transpose(0, 2, 1).reshape(B, CK, 128, NSLICE, TS)
        return np.ascontiguousarray(xt.transpose(0, 3, 2, 1, 4)).astype(hdt)

    xqT = _xdev(x_q)
    xkT = _xdev(x_kv)

    in_maps = [
        {"xq": xqT[b], "xk": xkT[b], "wq": wq, "wkv": wkv} for b in range(B)
    ]
    nc = _get_nc()
    res = run_bass_kernel_spmd(nc, in_maps, core_ids=list(range(B)), trace=_trace)
    # oT: [65, T]; rows 0:64 unnormalized out^T, row 64 softmax denominator
    outs = []
    for r in res.results:
        o = r["oT"]
        outs.append(np.ascontiguousarray((o[0:HS] / o[HS : HS + 1]).T))
    out = np.stack(outs).astype(np.float32)
    if _trace:
        kernel.last_result = res
    return out
